# revision 41
# baseline (speedup 1.0000x reference)
"""Trainium2 Bass kernel for nn_CyclicAttention (dense transformer attention layer).

Full computation:
    Q = q @ Wq + bq ; K = k @ Wk + bk ; V = v @ Wv + bv          [B,S,H]
    per head (DK=64): scores = Q K^T / 8 ; P = softmax(scores)
    attn = P V ; merged = concat heads                            [B,S,H]
    h = merged @ Wo + bo ; c = merged @ Wc + bc
    returns (c, h)

Sharding: 2D tensor-parallel - core (g, s) with g in 0..3 (head group: 4
heads = 256 projection columns) and s in 0..1 (batch). Each core reads only
its batch's host-transposed activations qT/kT/vT [H, S] (bf16), its
256-column slices of Wq/Wk/Wv and 256-row slices of Wo/Wc; host sums the 4
head-group output partials per batch, adds bo/bc, transposes.

Schedule (PE-bound; per-core PE floor ~178us of matmul at bf16):
  head: the first weight/activation DMA pieces are split small and
    interleaved so matmul 0 starts ~2.5us in; each K-proj chunk is emitted
    one chunk ahead of the scores group that consumes it (bias-add latency
    hides under a full chunk of PE filler); V-proj chunks slot under the
    (0,1) scores stream. The head is DMA-bandwidth-paced (~10.5 MB before
    pv(0,0)), so PE work is ordered to match byte arrival.
  steady state per query chunk qc: both score halves (qc,0) and (qc,1) are
    emitted BEFORE their pv blocks, with the two outproj halves (m=h / m=c)
    of chunk qc-1 as PE filler between them - exps for (qc,1) start ~7us
    earlier and pv(qc,1) never waits on ACT.
  softmax: exp on ACT, with 2 k-tiles per 16-tile block on DVE as a
    one-instruction int16 Schraudolph (bitcast bf16 ~ exp(s/8)); head blocks
    alternate ACT/DVE (both engines are otherwise idle there).
  normalize: exact DVE reciprocal of the ones-column denominators (the
    reciprocal_approx_* custom-DVE ops return garbage on HW), Pool
    partition_broadcast, DVE multiply; both hh chains emitted recip/recip,
    bcast/bcast, mult/mult so they pipeline instead of serializing.
  engines: ACT = exps + half the outproj PSUM->SBUF copies; DVE = the other
    copies, bias adds, V-bias, normalize; Pool = broadcasts + the m=0 output
    DMA dispatch (SWDGE) - Pool cannot read PSUM, and Pool-issued DMAs cost
    ~1us of Pool engine time, so output DMA dispatch alternates SP HWDGE /
    Pool SWDGE to parallelize the flush. Output stages are j-paired
    ([P,2,512] tiles, one DMA per pair) to halve dispatch count; the tail
    outproj uses wide 2-bank PSUM tiles from the (idle) scores pool so PSUM
    recycling never gates the drain.
"""

import numpy as np

H = 1024
NH = 16
DK = 64
C = 1024
B = 2
S = 2048
T = B * S
NCORES = 8
NG = 4             # head groups
CPC = H // NG      # 256 cols (4 heads) per core
P = 128
TCH = 512          # matmul moving-dim chunk
NHC = H // P       # 8 contraction chunks for projections
SKT = S // P       # 16 key tiles
SQC = S // TCH     # 4 query chunks

MM_DTYPE = "bfloat16"

import os as _os
# reciprocal_approx_fast returns garbage on HW (CoreSim models it
# as exact) — keep the exact DVE reciprocal
FAST_RECIP = _os.environ.get("K_FAST_RECIP", "0") == "1"
FUSED_EXP = _os.environ.get("K_FUSED_EXP", "1") == "1"
VDIRECT = _os.environ.get("K_VDIRECT", "1") == "1"
# Number of k-tiles per 16-tile scores block whose exp runs on DVE as a
# one-instruction bf16 Schraudolph (bitcast_bf16(int16(s*A + B)) ~ exp(s/8))
# instead of on ACT — balances the two engines' streaming load.
DVE_EXP_N = int(_os.environ.get("K_DVE_EXP", "2"))
# exp(s/8) = 2^(s * 0.125*log2(e)); bf16 bits = 128*(exponent+127 + frac)
SCH_A = 0.125 * 128 * 1.4426950408889634   # 23.0831206
SCH_B = 16256.0 - 5.7664 + 0.5             # minimax C, trunc-to-int adjust

_CACHE = {}


def _np_io_dtype(mm_dtype):
    if mm_dtype == "bfloat16":
        import ml_dtypes
        return np.dtype(ml_dtypes.bfloat16)
    return np.dtype(np.float32)


def _build_program(loop_n=None, mm_dtype=MM_DTYPE):
    import contextlib

    import concourse.tile as tile
    from concourse import bacc, mybir

    fp32 = mybir.dt.float32
    mdt = getattr(mybir.dt, mm_dtype)
    Act = mybir.ActivationFunctionType

    nc = bacc.Bacc("TRN2", target_bir_lowering=False, debug=False, num_devices=NCORES)

    # activations pre-tiled on host: [partition, chunk th, h-chunk, t]
    qT = nc.dram_tensor("qT", [P, SQC, NHC, TCH], mdt, kind="ExternalInput").ap()
    kT = nc.dram_tensor("kT", [P, SQC, NHC, TCH], mdt, kind="ExternalInput").ap()
    vT = nc.dram_tensor("vT", [P, SQC, NHC, TCH], mdt, kind="ExternalInput").ap()
    # weights pre-arranged on host to exact SBUF layouts (contiguous DMA)
    wq = nc.dram_tensor("wq", [P, NHC * 2 * P], mdt, kind="ExternalInput").ap()
    wk = nc.dram_tensor("wk", [P, NHC * 2 * P], mdt, kind="ExternalInput").ap()
    wv = nc.dram_tensor("wv", [P, NHC * CPC], mdt, kind="ExternalInput").ap()
    wo = nc.dram_tensor("wo", [P, 2 * H], mdt, kind="ExternalInput").ap()
    wc = nc.dram_tensor("wc", [P, 2 * C], mdt, kind="ExternalInput").ap()
    bq = nc.dram_tensor("bq", [P, 2], fp32, kind="ExternalInput").ap()
    bk = nc.dram_tensor("bk", [P, 2], fp32, kind="ExternalInput").ap()
    bv = nc.dram_tensor("bv", [1, CPC], fp32, kind="ExternalInput").ap()
    hT = nc.dram_tensor("hT", [H, S], mdt, kind="ExternalOutput").ap()
    cT = nc.dram_tensor("cT", [C, S], mdt, kind="ExternalOutput").ap()

    with tile.TileContext(nc) as tc:
        with (
            tc.tile_pool(name="const", bufs=1) as const,
            tc.tile_pool(name="wqkv", bufs=1) as wpool,
            tc.tile_pool(name="acts", bufs=1) as acts,
            tc.tile_pool(name="xin", bufs=4) as xin,
            tc.tile_pool(name="vin", bufs=2) as vin,
            tc.tile_pool(name="pt", bufs=40) as ptp,
            tc.tile_pool(name="small", bufs=4) as small,
            tc.tile_pool(name="ostage", bufs=6) as ostage,
            tc.tile_pool(name="ps_sp", bufs=2, space="PSUM") as ps_sp,
            tc.tile_pool(name="ps_pb", bufs=2, space="PSUM") as ps_pb,
            tc.tile_pool(name="ps_ap", bufs=2, space="PSUM") as ps_ap,
            tc.For_i(0, loop_n, 1, hint_engines=(
                mybir.EngineType.PE, mybir.EngineType.Activation,
                mybir.EngineType.DVE, mybir.EngineType.SP,
                mybir.EngineType.Pool,
            ), staggered_reset=True) if loop_n else contextlib.nullcontext(),
        ):
            # ---- constants ----
            # q/k weight slices as [h-chunk part, h-chunk idx, col-half, 128]
            # (wk first: K projects first so scores can start earliest; the
            # first 2 h-chunks ship separately so matmul 0 starts sooner)
            wk_sb = wpool.tile([P, NHC, 2, P], mdt, tag="wk")
            nc.sync.dma_start(
                wk_sb[:, 0:2].rearrange("p a u c -> p (a u c)"), wk[:, 0:2 * 2 * P])
            # first kT activation piece goes out right behind the first weight
            # piece (HWDGE dispatch serializes ~625ns per DMA — order matters)
            x_pre = xin.tile([P, 2, TCH], mdt, tag="x")
            nc.sync.dma_start(x_pre[:], kT[:, 0, 0:2, :])
            nc.sync.dma_start(
                wk_sb[:, 2:].rearrange("p a u c -> p (a u c)"), wk[:, 2 * 2 * P:])
            wq_sb = wpool.tile([P, NHC, 2, P], mdt, tag="wq")
            nc.sync.dma_start(wq_sb[:].rearrange("p a u c -> p (a u c)"), wq[:, :])
            if VDIRECT:
                # v weights (natural, moving operand) — DMA deferred off the
                # lead-in, emitted right before the V chunks
                wv_sb = wpool.tile([P, NHC, CPC], mdt, tag="wv")
                wvt_sb = ident = None
            else:
                from concourse.masks import make_identity
                wvt_sb = wpool.tile([P, NHC, 2, P], mdt, tag="wv")
                nc.sync.dma_start(
                    wvt_sb[:], wv.rearrange("(a p) (u c) -> p a u c", p=P, c=P))
                ident_f = const.tile([P, P], fp32, tag="identf")
                make_identity(nc, ident_f[:])
                ident = const.tile([P, P], mdt, tag="ident")
                nc.scalar.activation(ident[:], ident_f[:], Act.Copy)
            bk_sb = const.tile([P, 2], fp32, tag="bk")
            nc.sync.dma_start(bk_sb[:], bk[:, :])
            bq_sb = const.tile([P, 2], fp32, tag="bq")
            nc.sync.dma_start(bq_sb[:], bq[:, :])
            bv_row = const.tile([1, CPC], fp32, tag="bvr")
            bv_bc = const.tile([P, CPC], fp32, tag="bvb")

            # ---- persistent activations ----
            qTs = [[acts.tile([P, TCH], mdt, tag=f"qTs{u}_{t}", name=f"qT{u}_{t}")
                    for t in range(SQC)] for u in range(2)]
            kTs = [[acts.tile([P, TCH], mdt, tag=f"kTs{u}_{t}", name=f"kT{u}_{t}")
                    for t in range(SQC)] for u in range(2)]
            # V natural per k-tile: [128 keys, 4 heads, 64 dims + ones col]
            vh = [acts.tile([P, 4, 65], mdt, tag=f"vh{t}", name=f"vh{t}")
                  for t in range(SKT)]
            mTs = [acts.tile([P, S], mdt, tag=f"mTs{u}", name=f"mT{u}") for u in range(2)]

            for t in range(SKT):
                nc.vector.memset(vh[t][:, :, 64:65], 1.0)

            # One x stream feeds both column-halves (2 concurrent PSUM halves).
            # Each input DMA carries `groups` h-chunks via a 3D access pattern;
            # the very first chunk uses smaller leading pieces to cut the
            # cold-start DMA latency before matmul 0.
            def project_chunk(src_, w_sb, dsts, bias_sb, th, groups=(4, 4),
                              pre=None, pre_g=0):
                """pre: an already-DMA'd x tile covering the first pre_g
                h-chunks of this th (head-latency trimming)."""
                pss = ps_sp.tile([P, 2 * TCH], fp32, tag="sp", name=f"psp{th}")
                pieces = []
                if pre is not None:
                    pieces.append((pre, pre_g))
                hc_dma = pre_g
                for g in groups:
                    x = xin.tile([P, g, TCH], mdt, tag="x")
                    nc.sync.dma_start(x[:], src_[:, th, hc_dma:hc_dma + g, :])
                    pieces.append((x, g))
                    hc_dma += g
                hc = 0
                for x, g in pieces:
                    for hi in range(g):
                        for u in range(2):
                            nc.tensor.matmul(
                                pss[:, u * TCH:(u + 1) * TCH],
                                lhsT=w_sb[:, hc, u, :], rhs=x[:, hi, :],
                                start=(hc == 0), stop=(hc == NHC - 1))
                        hc += 1
                for u in range(2):
                    nc.vector.tensor_scalar_add(
                        dsts[u][th][:], pss[:, u * TCH:(u + 1) * TCH],
                        bias_sb[:, u:u + 1])

            # V natural direct: vT blocks stationary, Wv columns moving.
            def project_v_chunk(th):
                vts = vin.tile([P, NHC, TCH], mdt, tag="vts")
                nc.sync.dma_start(vts[:], vT[:, th, :, :])
                for i in range(TCH // P):
                    kt = th * (TCH // P) + i
                    pv = ps_pb.tile([P, CPC], fp32, tag="pb", name=f"pvn{kt}")
                    for hc in range(NHC):
                        nc.tensor.matmul(
                            pv[:], lhsT=vts[:, hc, i * P:(i + 1) * P],
                            rhs=wv_sb[:, hc, :],
                            start=(hc == 0), stop=(hc == NHC - 1))
                    nc.vector.tensor_tensor(
                        vh[kt][:, :, 0:64],
                        pv[:].rearrange("p (h d) -> p h d", d=64),
                        bv_bc[:].rearrange("p (h d) -> p h d", d=64),
                        op=mybir.AluOpType.add)

            # fallback V path (K_VDIRECT=0): V.T projection + PE transposes
            def project_v_chunk_transpose(th, wvt_sb, ident):
                tw = slice(th * TCH, (th + 1) * TCH)
                src3 = vT.rearrange("(a p) t -> p a t", p=P)
                vTs = [None, None]
                pss = ps_sp.tile([P, 2 * TCH], fp32, tag="sp", name=f"psv{th}")
                for hg in range(NHC // 4):
                    x = xin.tile([P, 4, TCH], mdt, tag="x")
                    nc.sync.dma_start(x[:], src3[:, hg * 4:hg * 4 + 4, tw])
                    for hi in range(4):
                        hc = hg * 4 + hi
                        for u in range(2):
                            nc.tensor.matmul(
                                pss[:, u * TCH:(u + 1) * TCH],
                                lhsT=wvt_sb[:, hc, u, :], rhs=x[:, hi, :],
                                start=(hc == 0), stop=(hc == NHC - 1))
                for u in range(2):
                    vTs[u] = vin.tile([P, TCH], mdt, tag=f"vTs{u}")
                    nc.vector.tensor_copy(vTs[u][:], pss[:, u * TCH:(u + 1) * TCH])
                for u in range(2):
                    for i in range(TCH // P):
                        kt = th * (TCH // P) + i
                        tp = ps_pb.tile([P, TCH], fp32, tag="pb", name="tpv")
                        tpv = tp[:, 0:P].bitcast(mdt)
                        nc.tensor.transpose(tpv, vTs[u][:, i * P:(i + 1) * P], ident[:])
                        for hh in range(2):
                            h = 2 * u + hh
                            nc.vector.tensor_tensor(
                                vh[kt][:, h, 0:64],
                                tpv[:, hh * 64:(hh + 1) * 64],
                                bv_bc[:, h * 64:(h + 1) * 64],
                                op=mybir.AluOpType.add)

            # ACT is kept exp-only: PSUM->SBUF copies go to Pool (hT) / DVE
            # (cT); all output DMAs dispatch via SP's HWDGE (a Pool-issued DMA
            # costs ~1us of Pool engine time and starves partition_broadcast).
            def out_projections(qc, us=(0, 1), ms=(0, 1), outs=None,
                                wide_po=False):
                qw = slice(qc * TCH, (qc + 1) * TCH)
                outTs = outs if outs is not None else (hT, cT)
                n = 0
                for jp in range(H // P // 2):
                    for m in ms:
                        w_sb, outT = ((wo_sb, outTs[0]), (wc_sb, outTs[1]))[m]
                        ot = ostage.tile([P, 2, TCH], mdt, tag="ot", name="ot")
                        if wide_po:
                            # tail-only: scores pool is free; a [P,1024] tile
                            # takes 4 matmuls per copy so PSUM recycling never
                            # gates PE at the drain
                            po2 = ps_sp.tile([P, 2 * TCH], fp32, tag="sp",
                                             name="po2")
                        for jj in range(2):
                            j = 2 * jp + jj
                            if wide_po:
                                po = po2[:, jj * TCH:(jj + 1) * TCH]
                            else:
                                pot = ps_pb.tile([P, TCH], fp32, tag="pb",
                                                 name="po")
                                po = pot[:]
                            for i, u in enumerate(us):
                                nc.tensor.matmul(
                                    po, lhsT=w_sb[:, u, j * P:(j + 1) * P],
                                    rhs=mTs[u][:, qw], start=(i == 0),
                                    stop=(i == len(us) - 1))
                            if not wide_po:
                                # copies alternate DVE/ACT (Pool can't read PSUM)
                                if n % 2 == 0:
                                    nc.vector.tensor_copy(ot[:, jj], po)
                                else:
                                    nc.scalar.activation(ot[:, jj], po, Act.Copy)
                                n += 1
                        if wide_po:
                            if n % 2 == 0:
                                nc.vector.tensor_copy(
                                    ot[:].rearrange("p a t -> p (a t)"), po2[:])
                            else:
                                nc.scalar.activation(
                                    ot[:].rearrange("p a t -> p (a t)"), po2[:],
                                    Act.Copy)
                            n += 1
                        # one paired DMA per (j-pair, m); dispatch alternates
                        # SP HWDGE / Pool SWDGE so the tail flush parallelizes
                        jw = slice(2 * jp * P, (2 * jp + 2) * P)
                        dst = (outT[jw, qw] if outs is None else outT[jw, :])
                        # dst rows are (a p): j-block index a=2 outer, then the
                        # 128 partition rows — match the [P, 2, TCH] source
                        dst3 = dst.rearrange("(a p) t -> p a t", a=2)
                        if m == 0:
                            nc.gpsimd.dma_start(dst3, ot[:])
                        else:
                            nc.sync.dma_start(dst3, ot[:])

            # k-tiles within each 16-tile scores block whose exp runs on DVE
            dve_kts = set()
            if DVE_EXP_N:
                step = SKT // DVE_EXP_N
                dve_kts = {step // 2 + i * step for i in range(DVE_EXP_N)}

            def scores_block(qc, u, kts=None, dve=None):
                pts = []
                for kt in kts if kts is not None else range(SKT):
                    use_dve = (kt in dve) if dve is not None else (kt in dve_kts)
                    kth, ki = kt // 4, kt % 4
                    kwi = slice(ki * P, (ki + 1) * P)
                    sp = ps_sp.tile([P, 2 * TCH], fp32, tag="sp", name="sp")
                    # scores.T for the pair (row-tiled: PE rows 0-63 / 64-127)
                    nc.tensor.matmul(sp[:, 0:TCH], lhsT=kTs[u][kth][0:64, kwi],
                                     rhs=qTs[u][qc][0:64, :], start=True, stop=True)
                    nc.tensor.matmul(sp[:, TCH:2 * TCH], lhsT=kTs[u][kth][64:128, kwi],
                                     rhs=qTs[u][qc][64:128, :], start=True, stop=True)
                    if use_dve:
                        pt = ptp.tile([P, 2 * TCH], mybir.dt.int16, tag="pt",
                                      name="pt")
                        nc.vector.tensor_scalar(
                            pt[:], sp[:], SCH_A, SCH_B,
                            op0=mybir.AluOpType.mult, op1=mybir.AluOpType.add)
                        pts.append((pt, True))
                        continue
                    pt = ptp.tile([P, 2 * TCH], mdt, tag="pt", name="pt")
                    if FUSED_EXP:
                        nc.scalar.activation(pt[:], sp[:], Act.Exp, scale=0.125)
                    else:
                        nc.scalar.activation(pt[:, 0:TCH], sp[:, 0:TCH],
                                             Act.Exp, scale=0.125)
                        nc.scalar.activation(pt[:, TCH:2 * TCH], sp[:, TCH:2 * TCH],
                                             Act.Exp, scale=0.125)
                    pts.append((pt, False))
                return pts

            def pv_block(qc, u, pts, kts=None, aps=None, finish=True):
                qw = slice(qc * TCH, (qc + 1) * TCH)
                if aps is None:
                    ap0 = ps_ap.tile([65, TCH], fp32, tag="ap", name="ap0")
                    ap1 = ps_ap.tile([65, TCH], fp32, tag="ap", name="ap1")
                else:
                    ap0, ap1 = aps
                for kt in kts if kts is not None else range(SKT):
                    pt, is_i16 = pts[kt]
                    p0 = pt[:, 0:TCH].bitcast(mdt) if is_i16 else pt[:, 0:TCH]
                    p1 = (pt[:, TCH:2 * TCH].bitcast(mdt) if is_i16
                          else pt[:, TCH:2 * TCH])
                    nc.tensor.matmul(ap0[:], lhsT=vh[kt][:, 2 * u, :],
                                     rhs=p0,
                                     start=(kt == 0), stop=(kt == SKT - 1))
                    nc.tensor.matmul(ap1[:], lhsT=vh[kt][:, 2 * u + 1, :],
                                     rhs=p1,
                                     start=(kt == 0), stop=(kt == SKT - 1))
                if not finish:
                    return (ap0, ap1)
                # both recips first, then both broadcasts, then both mults —
                # interleaving per-hh serializes the two chains on DVE's queue
                recs, rbcs = [], []
                for hh, ap in ((0, ap0), (1, ap1)):
                    rec = small.tile([1, TCH], fp32, tag="rec")
                    if FAST_RECIP:
                        nc.vector.reciprocal(rec[:], ap[64:65, :])
                    else:
                        nc.vector.reciprocal(rec[:], ap[64:65, :])
                    recs.append(rec)
                for hh in (0, 1):
                    rbc = small.tile([64, TCH], fp32, tag="rbc")
                    nc.gpsimd.partition_broadcast(rbc[:], recs[hh][:])
                    rbcs.append(rbc)
                for hh, ap in ((0, ap0), (1, ap1)):
                    nc.vector.tensor_tensor(
                        mTs[u][hh * 64:(hh + 1) * 64, qw], ap[0:64, :],
                        rbcs[hh][:], op=mybir.AluOpType.mult)

            # ---- emission order: interleave the first chunk's score k-tiles
            # with the K projection chunks so exp starts as early as possible;
            # V and remaining Q projections slot in under the first exps;
            # outproj(qc-1) is PE filler while ACT streams chunk qc's exps ----
            project_chunk(kT, wk_sb, kTs, bk_sb, 0, groups=(2, 4),
                          pre=x_pre, pre_g=2)
            project_chunk(qT, wq_sb, qTs, bq_sb, 0)
            # head blocks: ACT is otherwise idle and DVE nearly so — alternate
            # exps between them so the 2-buf score PSUM drains at PE pace.
            # Each scores group runs a full proj chunk AFTER the one feeding
            # it, giving the Pool bias-add latency PE filler to hide under.
            _hd = int(_os.environ.get("K_HEAD_DVE", "2"))
            head_dve = set(range(1, SKT, _hd))
            project_chunk(kT, wk_sb, kTs, bk_sb, 1)
            if VDIRECT:
                nc.sync.dma_start(
                    wv_sb[:].rearrange("p a j -> p (a j)"), wv[:, :])
            nc.sync.dma_start(bv_row[:], bv[:, :])
            nc.gpsimd.partition_broadcast(bv_bc[:], bv_row[:])
            p00 = scores_block(0, 0, kts=range(0, 4), dve=head_dve)
            project_chunk(kT, wk_sb, kTs, bk_sb, 2)
            p00 += scores_block(0, 0, kts=range(4, 8), dve=head_dve)
            project_chunk(kT, wk_sb, kTs, bk_sb, 3)
            p00 += scores_block(0, 0, kts=range(8, 12), dve=head_dve)
            if VDIRECT:
                project_v_chunk(0)
            else:
                project_v_chunk_transpose(0, wvt_sb, ident)
            p00 += scores_block(0, 0, kts=range(12, 16), dve=head_dve)
            p01 = []
            for th in range(1, SQC):
                if VDIRECT:
                    project_v_chunk(th)
                else:
                    project_v_chunk_transpose(th, wvt_sb, ident)
                p01 += scores_block(0, 1, kts=range(4 * (th - 1), 4 * th),
                                    dve=head_dve)
            # output weights [c-chunk part, col half u, out-col] — deferred
            # off the critical lead-in (first used by outproj(0))
            wo_sb = wpool.tile([P, 2, H], mdt, tag="wo")
            nc.sync.dma_start(wo_sb[:].rearrange("p a j -> p (a j)"), wo[:, :])
            wc_sb = wpool.tile([P, 2, C], mdt, tag="wc")
            nc.sync.dma_start(wc_sb[:].rearrange("p a j -> p (a j)"), wc[:, :])
            p01 += scores_block(0, 1, kts=range(4 * (SQC - 1), SKT),
                                dve=head_dve)
            pv_block(0, 0, p00)
            pv_block(0, 1, p01)
            # steady state: both score halves are emitted BEFORE their pv
            # blocks (exps for u1 start ~7us earlier, so pv(qc,1) never waits
            # on ACT); the two outproj halves of the previous chunk are the
            # PE filler between them.
            for qc in range(1, SQC):
                project_chunk(qT, wq_sb, qTs, bq_sb, qc)
                pu0 = scores_block(qc, 0)
                out_projections(qc - 1, ms=(0,))
                pu1 = scores_block(qc, 1)
                out_projections(qc - 1, ms=(1,))
                pv_block(qc, 0, pu0)
                pv_block(qc, 1, pu1)
            out_projections(SQC - 1, wide_po=True)

    nc.compile()
    return nc


def _get_program():
    if "nc" not in _CACHE:
        _CACHE["nc"] = _build_program()
    return _CACHE["nc"]


def _tile_act(xT, iodt):
    # [H, S] -> [P, SQC, NHC, TCH]: (p, th, a, t) = xT[a*128+p, th*512+t]
    r = xT.reshape(NHC, P, SQC, TCH).transpose(1, 2, 0, 3)
    return np.ascontiguousarray(r).astype(iodt)


def _tile_w_in(W, iodt):
    # [H, CPC] -> [P, NHC*2*P]: (p, a, u, c) = W[a*128+p, u*128+c]
    r = W.reshape(NHC, P, 2, P).transpose(1, 0, 2, 3).reshape(P, NHC * 2 * P)
    return np.ascontiguousarray(r).astype(iodt)


def _tile_w_nat(W, iodt):
    # [H, CPC] -> [P, NHC*CPC]: (p, a, j) = W[a*128+p, j]
    r = W.reshape(NHC, P, CPC).transpose(1, 0, 2).reshape(P, NHC * CPC)
    return np.ascontiguousarray(r).astype(iodt)


def _tile_w_out(W, iodt):
    # [CPC, n] -> [P, 2*n]: (p, a, j) = W[a*128+p, j]
    n = W.shape[1]
    r = W.reshape(2, P, n).transpose(1, 0, 2).reshape(P, 2 * n)
    return np.ascontiguousarray(r).astype(iodt)


def make_in_maps(q, k, v, Wq, bq, Wk, bk, Wv, bv, Wo, bo, Wc, bc, mm_dtype=MM_DTYPE):
    iodt = _np_io_dtype(mm_dtype)
    q = np.asarray(q, np.float32).reshape(T, H)
    k = np.asarray(k, np.float32).reshape(T, H)
    v = np.asarray(v, np.float32).reshape(T, H)
    # per-batch transposed activations, pre-tiled [P, SQC, NHC, TCH]
    qTb = [_tile_act(q[s * S:(s + 1) * S].T, iodt) for s in range(B)]
    kTb = [_tile_act(k[s * S:(s + 1) * S].T, iodt) for s in range(B)]
    vTb = [_tile_act(v[s * S:(s + 1) * S].T, iodt) for s in range(B)]
    wqg, wkg, wvg, wog, wcg, bqg, bkg, bvg = [], [], [], [], [], [], [], []
    for g in range(NG):
        cs = slice(g * CPC, (g + 1) * CPC)
        wqg.append(_tile_w_in(np.asarray(Wq, np.float32)[:, cs], iodt))
        wkg.append(_tile_w_in(np.asarray(Wk, np.float32)[:, cs], iodt))
        if VDIRECT:
            wvg.append(_tile_w_nat(np.asarray(Wv, np.float32)[:, cs], iodt))
        else:
            wvg.append(np.ascontiguousarray(
                np.asarray(Wv, np.float32)[:, cs]).astype(iodt))
        wog.append(_tile_w_out(np.asarray(Wo, np.float32)[cs, :], iodt))
        wcg.append(_tile_w_out(np.asarray(Wc, np.float32)[cs, :], iodt))
        bqg.append(np.asarray(bq, np.float32)[cs].reshape(2, P).T.copy())
        bkg.append(np.asarray(bk, np.float32)[cs].reshape(2, P).T.copy())
        bvg.append(np.asarray(bv, np.float32)[cs].reshape(1, CPC).copy())

    in_maps = []
    for core in range(NCORES):
        g, s = core % NG, core // NG
        in_maps.append({
            "qT": qTb[s], "kT": kTb[s], "vT": vTb[s],
            "wq": wqg[g], "wk": wkg[g], "wv": wvg[g],
            "wo": wog[g], "wc": wcg[g],
            "bq": bqg[g], "bk": bkg[g], "bv": bvg[g],
        })
    return in_maps


def combine_outputs(results, bo, bc):
    h = np.zeros((B, S, H), np.float32)
    cc = np.zeros((B, S, C), np.float32)
    for s in range(B):
        hT_full = np.zeros((H, S), np.float64)
        cT_full = np.zeros((C, S), np.float64)
        for g in range(NG):
            core = s * NG + g
            hT_full += np.asarray(results[core]["hT"], np.float64)
            cT_full += np.asarray(results[core]["cT"], np.float64)
        h[s] = hT_full.T.astype(np.float32) + np.asarray(bo, np.float32)
        cc[s] = cT_full.T.astype(np.float32) + np.asarray(bc, np.float32)
    return (cc, h)


def kernel(q, k, v, Wq, bq, Wk, bk, Wv, bv, Wo, bo, Wc, bc):
    from concourse.bass_utils import run_bass_kernel_spmd

    nc = _get_program()
    in_maps = make_in_maps(q, k, v, Wq, bq, Wk, bk, Wv, bv, Wo, bo, Wc, bc)
    res = run_bass_kernel_spmd(nc, in_maps, core_ids=list(range(NCORES)))
    _CACHE["last_results"] = res
    return combine_outputs(res.results, bo, bc)

